# revision 34
# baseline (speedup 1.0000x reference)
"""Multi-head attention (B=4, N=2048, D=1024, H=16) on 8 TRN2 NeuronCores.

Sharding: 8 cores = batch(4) x sequence-half(2). Each core computes the full
attention output for its 1024-token slice of one batch (all 16 heads); the
only cross-core traffic is a pairwise AllGather of K^T and V.

Per-core pipeline (bf16 matmul operands, fp32 PSUM accumulation):
  1. Prologue: fp32 loads stream on TWO queues (x on scalar, weights on
     sync) at ~150GB/s; casts to bf16 on ScalarE; transposes ON THE PE
     (identity matmul, bf16, PSUM->SBUF copies on Pool) - no DRAM staging
     round trip. Only w_proj keeps a DMA-transpose round trip, executed in
     the background during attention.
  2. K projection -> AllGather K^T. V projection -> AllGather V in four
     row-quarters so the first attention unit's O matmuls aren't gated on
     the full gather. Q projection m-tiles interleave with attention units
     (their PSUM->SBUF copies run on Pool so DVE recips can't block them).
  3. Attention per (head-pair p, q-half qc): S^T row-paired (contraction
     64), exp on ScalarE, O^T against a ones-AUGMENTED V (65th lhsT column
     = 1.0) so the softmax denominator accumulates in PSUM row 64 of the
     same chain - no denominator matmuls. k-tiles visit in V-gather
     arrival order.
  4. Normalize: denominator rows copied (Pool) into one tile, single DVE
     reciprocal, DRAM-bounce broadcast to 64 partitions, one DVE multiply
     per head (partition-base mismatch between out and ins is fine).
  5. Output projection per qc half, interleaved into the other half's units.
"""

import sys

for _p in ("/opt/trn_rl_repo",):
    if _p not in sys.path:
        sys.path.insert(0, _p)

import numpy as np

import concourse.bass as bass
import concourse.masks as masks
import concourse.mybir as mybir
import concourse.tile as tile
from concourse import bacc
from concourse.bass_utils import run_bass_kernel_spmd

B, N, D, H, HD = 4, 2048, 1024, 16, 64
SCALE = HD ** -0.5
NL = N // 2  # tokens per core
NCORES = 8
RG = [[0, 1], [2, 3], [4, 5], [6, 7]]
F32 = mybir.dt.float32
BF16 = mybir.dt.bfloat16
EXP = mybir.ActivationFunctionType.Exp
VW = 66  # vv row pitch (64 v + 1 ones + 1 pad for 4B alignment)
# k-tiles in V-gather quarter arrival order (quarter q covers local tokens
# 256q..256q+255 of both halves -> global k-tiles {2q, 2q+1, 2q+8, 2q+9})
KT_ORDER = [0, 1, 8, 9, 2, 3, 10, 11, 4, 5, 12, 13, 6, 7, 14, 15]


def _emit(tc, aps):
    nc = tc.nc
    x_l, wqkv, wproj, bias, out = (
        aps["x_local"], aps["w_qkv"], aps["w_proj"], aps["b_proj"], aps["out"])
    wpbf = aps["wpbf"]
    cc_k, cc_v, k_g, v_g = aps["cc_k"], aps["cc_v"], aps["k_g"], aps["v_g"]
    scratch = aps["scratch"]

    persist = tc.alloc_tile_pool(name="persist", bufs=1)

    bias_sb = persist.tile([128, D], F32, tag="bias")
    bias_bcast = bass.AP(tensor=bias.tensor, offset=bias.offset,
                         ap=[[0, 128], *bias.ap])
    nc.sync.dma_start(out=bias_sb, in_=bias_bcast)

    ident = persist.tile([128, 128], BF16, tag="ident", name="ident")
    masks.make_identity(nc, ident[:])

    qTm = persist.tile([128, 8, NL], BF16, tag="qTm", name="qTm")
    kTm = persist.tile([128, 8, N], BF16, tag="kTm", name="kTm")
    wpTm = persist.tile([128, 8, D], BF16, tag="wpTm", name="wpTm")
    attoutT = persist.tile([128, 8, NL], BF16, tag="attoutT", name="attoutT")
    # ones-augmented V: per k-tile [128 ktok, 16 heads, 64 v | 1 ones | pad]
    vv = [persist.tile([128, H, VW], BF16, tag=f"vv{kt}", name=f"vv{kt}")
          for kt in range(16)]
    for kt in range(16):
        nc.gpsimd.memset(vv[kt][:, :, HD:HD + 1], 1.0)

    xt_pool = tc.alloc_tile_pool(name="xt", bufs=1)
    wt_pool = tc.alloc_tile_pool(name="wt", bufs=2)
    xTm = xt_pool.tile([128, 8, NL], BF16, tag="xTm", name="xTm")

    # transient prologue pools on the right stack so they release before the
    # attention pools allocate
    ld_pool = tc.alloc_tile_pool(name="ld", bufs=5, side="right")
    cast_pool = tc.alloc_tile_pool(name="cast", bufs=3, side="right")
    stage_pool = tc.alloc_tile_pool(name="stage", bufs=1, side="right")

    # K^T and V staging share one buffer: cc_k's store drains it before the
    # V projection copies land (WAR tracked by the pool).
    kstage = stage_pool.tile([128, 8, NL], BF16, tag="stage", name="kstage")

    # PSUM: ps (K/V proj) + tr (PE transposes) tags, 2 bufs each = 4 banks
    qkvps = tc.alloc_tile_pool(name="qkv_ps", bufs=2, space="PSUM")

    def load128(src, row0, ldq):
        # 128-row fp32 chunk on queue ldq
        ld = ld_pool.tile([128, D], F32, tag="ld", name="ld")
        ldq.dma_start(out=ld, in_=src[row0:row0 + 128, :])
        return ld

    def cast128(ld, on_dve=False):
        cb = cast_pool.tile([128, D], BF16, tag="cast", name="cb")
        if on_dve:
            nc.vector.tensor_copy(cb, ld)
        else:
            nc.scalar.copy(cb, ld)
        return cb

    def pe_transpose(cb, dstm, r0):
        # transpose a [128, D] bf16 row-tile into dstm[:, :, r0:r0+128] via
        # identity matmuls; one Pool copy moves PSUM -> SBUF
        trp = qkvps.tile([128, 8, 128], BF16, tag="tr", name="trp")
        for c in range(8):
            nc.tensor.matmul(
                out=trp[:, c, :],
                lhsT=cb[:, c * 128:(c + 1) * 128],
                rhs=ident,
                is_transpose=True)
        nc.vector.tensor_copy(dstm[:, :, r0:r0 + 128], trp)

    # x (scalar queue) and w_qkv K rows (sync queue) load concurrently with
    # interleaved buffer rotation; casts follow arrival order on ScalarE.
    # V and Q row loads are emitted behind them on sync; their casts and PE
    # transposes are deferred into the K/V projection phases below.
    x_cb, wk_cb = [], []
    for c in range(8):
        xl = load128(x_l, c * 128, nc.scalar)
        kl = load128(wqkv, D + c * 128, nc.sync)
        x_cb.append(cast128(xl))
        wk_cb.append(cast128(kl))
    wv_ld = [load128(wqkv, 2 * D + c * 128, nc.sync) for c in range(8)]
    wq_ld = [load128(wqkv, c * 128, nc.sync) for c in range(8)]
    # transposes interleave x/wK in the same order as the casts so the cast
    # pool's slot rotation never inverts against the in-order PE queue
    wtK = wt_pool.tile([128, 8, D], BF16, tag="wT", name="wtK")
    for c in range(8):
        pe_transpose(x_cb[c], xTm, c * 128)
        pe_transpose(wk_cb[c], wtK, c * 128)

    # ---- Phase B: K/V projections + collectives -------------------------
    def proj_kv(wt, m, dst, as_lhsT):
        pss = []
        for qc in range(2):
            ps = qkvps.tile([128, 512], F32, tag="ps", name="ps")
            pss.append(ps)
        for k in range(8):
            for qc in range(2):
                if as_lhsT:  # V: out[tok, e]
                    nc.tensor.matmul(
                        out=pss[qc],
                        lhsT=xTm[:, k, m * 128:(m + 1) * 128],
                        rhs=wt[:, k, qc * 512:(qc + 1) * 512],
                        start=(k == 0), stop=(k == 7))
                else:        # K: out[e, tok]
                    nc.tensor.matmul(
                        out=pss[qc],
                        lhsT=wt[:, k, m * 128:(m + 1) * 128],
                        rhs=xTm[:, k, qc * 512:(qc + 1) * 512],
                        start=(k == 0), stop=(k == 7))
        for qc in range(2):
            nc.vector.tensor_copy(dst[:, qc * 512:(qc + 1) * 512], pss[qc])

    # K projection; wV casts+transposes interleave after m=5
    wtV = wt_pool.tile([128, 8, D], BF16, tag="wT", name="wtV")
    for m in range(8):
        proj_kv(wtK, m, kstage[:, m, :], as_lhsT=False)
        if m == 5:
            for c in range(8):
                pe_transpose(cast128(wv_ld[c], on_dve=True), wtV, c * 128)
    # K^T AllGather in two m-halves so attention unit 0 isn't gated on the
    # full 2MB store+gather+load chain
    for kh in range(2):
        cck_dst = bass.AP(tensor=cc_k.tensor,
                          offset=cc_k.offset + kh * 4 * 128 * NL,
                          ap=[[NL, 128], [128 * NL, 4], [1, NL]])
        nc.gpsimd.dma_start(out=cck_dst, in_=kstage[:, 4 * kh:4 * kh + 4, :])
        nc.gpsimd.collective_compute(
            "AllGather", mybir.AluOpType.bypass, replica_groups=RG,
            ins=[cc_k[kh * 512:(kh + 1) * 512, :]], outs=[k_g[kh]])
        for half in range(2):
            src = bass.AP(tensor=k_g.tensor,
                          offset=k_g.offset + (kh * 2 + half) * 512 * NL,
                          ap=[[NL, 128], [128 * NL, 4], [1, NL]])
            nc.sync.dma_start(
                out=kTm[:, 4 * kh:4 * kh + 4, half * NL:(half + 1) * NL],
                in_=src)

    # V projection in four row-quarters; wQ transposes interleave after q=1
    wtQ = wt_pool.tile([128, 8, D], BF16, tag="wT", name="wtQ")
    vstage = stage_pool.tile([128, 8, D], BF16, tag="stage", name="vstage")
    for q in range(4):
        for t in range(2 * q, 2 * q + 2):
            proj_kv(wtV, t, vstage[:, t, :], as_lhsT=True)
        if q == 1:
            for c in range(8):
                pe_transpose(cast128(wq_ld[c], on_dve=True), wtQ, c * 128)
        ccv_dst = bass.AP(tensor=cc_v.tensor, offset=cc_v.offset + q * 256 * D,
                          ap=[[D, 128], [128 * D, 2], [1, D]])
        nc.gpsimd.dma_start(out=ccv_dst, in_=vstage[:, 2 * q:2 * q + 2, :])
        nc.gpsimd.collective_compute(
            "AllGather", mybir.AluOpType.bypass, replica_groups=RG,
            ins=[cc_v[q * 256:(q + 1) * 256, :]],
            outs=[v_g[q]])
        for half in range(2):
            for tl in range(2):
                kt = half * 8 + 2 * q + tl
                nc.sync.dma_start(
                    out=vv[kt][:, :, 0:HD],
                    in_=v_g[q, half, tl * 128:(tl + 1) * 128, :]
                        .rearrange("p (h c) -> p h c", h=H))

    # w_proj round trip (DMA-transpose) runs in the background during
    # attention: loads on sync, casts on DVE, stores gpsimd, transpose sync
    for c in range(8):
        ld = ld_pool.tile([128, D], F32, tag="ld", name="ld")
        nc.sync.dma_start(out=ld, in_=wproj[c * 128:(c + 1) * 128, :])
        cb = cast_pool.tile([128, D], BF16, tag="cast", name="cb")
        nc.vector.tensor_copy(cb, ld)
        nc.gpsimd.dma_start(out=wpbf[c * 128:(c + 1) * 128, :], in_=cb)
    nc.sync.dma_start_transpose(out=wpTm, in_=wpbf)

    qkvps.release()
    stage_pool.release()
    cast_pool.release()
    ld_pool.release()

    # ---- Phase C: Q projection interleaved with attention ---------------
    spool = tc.alloc_tile_pool(name="s_ps", bufs=2, space="PSUM")
    oapool = tc.alloc_tile_pool(name="oa_ps", bufs=1, space="PSUM")
    ptpool = tc.alloc_tile_pool(name="pt", bufs=4)
    stgpool = tc.alloc_tile_pool(name="stg", bufs=2)
    rcpool = tc.alloc_tile_pool(name="rc", bufs=2)
    rbcpool = tc.alloc_tile_pool(name="rbc", bufs=2)
    ytpool = tc.alloc_tile_pool(name="yt", bufs=1)

    def proj_q(m):
        ps = spool.tile([128, 2, 512], F32, tag="s", name="ps_q")
        for k in range(8):
            for qc in range(2):
                nc.tensor.matmul(
                    out=ps[:, qc, :],
                    lhsT=wtQ[:, k, m * 128:(m + 1) * 128],
                    rhs=xTm[:, k, qc * 512:(qc + 1) * 512],
                    start=(k == 0), stop=(k == 7))
        nc.vector.tensor_copy(qTm[:, m, :], ps.rearrange("p a b -> p (a b)"))

    def unit(p, qc):
        # O accumulates in two 8-long even/odd chains per head (shorter PSUM
        # accumulation chains run measurably faster per matmul); the merge
        # happens on DVE right after the stops, freeing the banks early.
        oaE = [oapool.tile([128, 512], F32, tag=f"oaE{h}", name=f"oaE{h}")
               for h in range(2)]
        oaO = [oapool.tile([128, 512], F32, tag=f"oaO{h}", name=f"oaO{h}")
               for h in range(2)]
        for i, kt in enumerate(KT_ORDER):
            s = spool.tile([128, 2, 512], F32, tag="s", name="s")
            for h in range(2):
                nc.tensor.matmul(
                    out=s[:, h, :],
                    lhsT=kTm[h * 64:(h + 1) * 64, p, kt * 128:(kt + 1) * 128],
                    rhs=qTm[h * 64:(h + 1) * 64, p, qc * 512:(qc + 1) * 512],
                    start=True, stop=True,
                    tile_position=(h * 64, 0))
            pt = ptpool.tile([128, 2, 512], BF16, tag="pt", name="pt")
            nc.scalar.activation(pt, s, EXP, scale=SCALE)
            dst = oaE if i % 2 == 0 else oaO
            for h in range(2):
                nc.tensor.matmul(
                    out=dst[h][0:HD + 1, :],
                    lhsT=vv[kt][:, 2 * p + h, 0:HD + 1],
                    rhs=pt[:, h, :],
                    start=(i < 2), stop=(i >= 14))
        # merge even/odd chains: numerator rows -> bf16 staging, denominator
        # row 64 -> f32; then reciprocal, DRAM-bounce broadcast, multiply
        u = qc * 8 + p
        stg = stgpool.tile([128, 2, 512], BF16, tag="stg", name="stg")
        rc = rcpool.tile([1, 2, 512], F32, tag="rc", name="rc")
        dn = rcpool.tile([1, 2, 512], F32, tag="dn", name="dn")
        for h in range(2):
            # DVE reads at most one PSUM operand per instruction
            nc.vector.tensor_copy(stg[0:HD, h, :], oaE[h][0:HD, :])
            nc.vector.tensor_add(stg[0:HD, h, :], stg[0:HD, h, :],
                                 oaO[h][0:HD, :])
            nc.vector.tensor_copy(dn[:, h, :], oaE[h][HD:HD + 1, :])
            nc.vector.tensor_add(dn[:, h, :], dn[:, h, :],
                                 oaO[h][HD:HD + 1, :])
        nc.vector.reciprocal(rc, dn)
        nc.gpsimd.dma_start(out=scratch[u], in_=rc)
        rbc = rbcpool.tile([64, 2, 512], F32, tag="rbc", name="rbc")
        rsrc = bass.AP(tensor=scratch.tensor,
                       offset=scratch.offset + u * 1024,
                       ap=[[0, 64], [512, 2], [1, 512]])
        nc.sync.dma_start(out=rbc, in_=rsrc)
        for h in range(2):
            nc.vector.tensor_mul(
                attoutT[h * 64:(h + 1) * 64, p, qc * 512:(qc + 1) * 512],
                stg[0:HD, h, :], rbc[:, h, :])

    def outproj(tt):
        yt = ytpool.tile([128, D], F32, tag="yt", name="yt")
        ps = spool.tile([128, 2, 512], F32, tag="s", name="ps_o")
        for p in range(8):
            for ec in range(2):
                nc.tensor.matmul(
                    out=ps[:, ec, :],
                    lhsT=attoutT[:, p, tt * 128:(tt + 1) * 128],
                    rhs=wpTm[:, p, ec * 512:(ec + 1) * 512],
                    start=(p == 0), stop=(p == 7))
        for ec in range(2):
            nc.vector.tensor_add(yt[:, ec * 512:(ec + 1) * 512], ps[:, ec, :],
                                 bias_sb[:, ec * 512:(ec + 1) * 512])
        nc.sync.dma_start(out=out[tt * 128:(tt + 1) * 128, :], in_=yt)

    # qc0 pass: Q projection m-tiles lead their consuming unit by two
    proj_q(0)
    proj_q(1)
    for p in range(8):
        if p + 2 < 8:
            proj_q(p + 2)
        unit(p, 0)
    # outproj(qc0 half) emitted two units into the qc1 pass so the PE never
    # waits on the last qc0 unit's normalize round trip
    for p in range(8):
        unit(p, 1)
        if p == 1:
            for tt in range(4):
                outproj(tt)
    for tt in range(4, 8):
        outproj(tt)

    ytpool.release()
    rbcpool.release()
    rcpool.release()
    stgpool.release()
    ptpool.release()
    oapool.release()
    spool.release()
    wt_pool.release()
    xt_pool.release()
    persist.release()


def _build():
    nc = bacc.Bacc("TRN2", target_bir_lowering=False, debug=False,
                   num_devices=NCORES)
    aps = {
        "x_local": nc.dram_tensor("x_local", [NL, D], F32, kind="ExternalInput").ap(),
        "w_qkv": nc.dram_tensor("w_qkv", [3 * D, D], F32, kind="ExternalInput").ap(),
        "w_proj": nc.dram_tensor("w_proj", [D, D], F32, kind="ExternalInput").ap(),
        "b_proj": nc.dram_tensor("b_proj", [D], F32, kind="ExternalInput").ap(),
        "out": nc.dram_tensor("out", [NL, D], F32, kind="ExternalOutput").ap(),
        "wpbf": nc.dram_tensor("wpbf", [D, D], BF16).ap(),
        "cc_k": nc.dram_tensor("cc_k", [D, NL], BF16).ap(),
        "cc_v": nc.dram_tensor("cc_v", [NL, D], BF16).ap(),
        "k_g": nc.dram_tensor("k_g", [2, 2, 512, NL], BF16).ap(),
        "v_g": nc.dram_tensor("v_g", [4, 2, 256, D], BF16).ap(),
        "scratch": nc.dram_tensor("scratch", [16, 2, 512], F32).ap(),
    }
    with tile.TileContext(nc) as tc:
        _emit(tc, aps)
    nc.compile()
    return nc


_NC = None


def _get_nc():
    global _NC
    if _NC is None:
        _NC = _build()
    return _NC


def run(x, w_qkv, w_proj, b_proj, **spmd_kwargs):
    nc = _get_nc()
    x = np.ascontiguousarray(np.asarray(x, dtype=np.float32))
    w_qkv = np.ascontiguousarray(np.asarray(w_qkv, dtype=np.float32))
    w_proj = np.ascontiguousarray(np.asarray(w_proj, dtype=np.float32))
    b_proj = np.ascontiguousarray(np.asarray(b_proj, dtype=np.float32))
    in_maps = []
    for c in range(NCORES):
        b, half = divmod(c, 2)
        in_maps.append({
            "x_local": np.ascontiguousarray(x[b, half * NL:(half + 1) * NL, :]),
            "w_qkv": w_qkv,
            "w_proj": w_proj,
            "b_proj": b_proj,
        })
    res = run_bass_kernel_spmd(nc, in_maps, list(range(NCORES)), **spmd_kwargs)
    y = np.empty((B, N, D), dtype=np.float32)
    for c in range(NCORES):
        b, half = divmod(c, 2)
        y[b, half * NL:(half + 1) * NL, :] = res.results[c]["out"]
    return y, res


def kernel(x, w_qkv, w_proj, b_proj):
    y, _ = run(x, w_qkv, w_proj, b_proj)
    return y


# revision 35
# speedup vs baseline: 1.1323x; 1.1323x over previous
"""Multi-head attention (B=4, N=2048, D=1024, H=16) on 8 TRN2 NeuronCores.

Sharding: 8 cores = batch(4) x sequence-half(2). Each core computes the full
attention output for its 1024-token slice of one batch (all 16 heads); the
only cross-core traffic is a pairwise AllGather of K^T and V.

Per-core pipeline (bf16 matmul operands, fp32 PSUM accumulation):
  1. Prologue: fp32 loads stream on TWO queues (x on scalar, weights on
     sync) at ~150GB/s; casts to bf16 on ScalarE; transposes ON THE PE
     (identity matmul, bf16, PSUM->SBUF copies on Pool) - no DRAM staging
     round trip. Only w_proj keeps a DMA-transpose round trip, executed in
     the background during attention.
  2. K projection -> AllGather K^T. V projection -> AllGather V in four
     row-quarters so the first attention unit's O matmuls aren't gated on
     the full gather. Q projection m-tiles interleave with attention units
     (their PSUM->SBUF copies run on Pool so DVE recips can't block them).
  3. Attention per (head-pair p, q-half qc): S^T row-paired (contraction
     64), exp on ScalarE, O^T against a ones-AUGMENTED V (65th lhsT column
     = 1.0) so the softmax denominator accumulates in PSUM row 64 of the
     same chain - no denominator matmuls. k-tiles visit in V-gather
     arrival order.
  4. Normalize: denominator rows copied (Pool) into one tile, single DVE
     reciprocal, DRAM-bounce broadcast to 64 partitions, one DVE multiply
     per head (partition-base mismatch between out and ins is fine).
  5. Output projection per qc half, interleaved into the other half's units.
"""

import sys

for _p in ("/opt/trn_rl_repo",):
    if _p not in sys.path:
        sys.path.insert(0, _p)

import numpy as np

import concourse.bass as bass
import concourse.masks as masks
import concourse.mybir as mybir
import concourse.tile as tile
from concourse import bacc
from concourse.bass_utils import run_bass_kernel_spmd

B, N, D, H, HD = 4, 2048, 1024, 16, 64
SCALE = HD ** -0.5
NL = N // 2  # tokens per core
NCORES = 8
RG = [[0, 1], [2, 3], [4, 5], [6, 7]]
F32 = mybir.dt.float32
BF16 = mybir.dt.bfloat16
EXP = mybir.ActivationFunctionType.Exp
VW = 66  # vv row pitch (64 v + 1 ones + 1 pad for 4B alignment)
# k-tiles in V-gather quarter arrival order (quarter q covers local tokens
# 256q..256q+255 of both halves -> global k-tiles {2q, 2q+1, 2q+8, 2q+9})
KT_ORDER = [0, 1, 8, 9, 2, 3, 10, 11, 4, 5, 12, 13, 6, 7, 14, 15]


def _emit(tc, aps):
    nc = tc.nc
    x_l, wqkv, wproj, bias, out = (
        aps["x_local"], aps["w_qkv"], aps["w_proj"], aps["b_proj"], aps["out"])
    wpbf = aps["wpbf"]
    cc_k, cc_v, k_g, v_g = aps["cc_k"], aps["cc_v"], aps["k_g"], aps["v_g"]
    scratch = aps["scratch"]

    persist = tc.alloc_tile_pool(name="persist", bufs=1)

    bias_sb = persist.tile([128, D], F32, tag="bias")
    bias_bcast = bass.AP(tensor=bias.tensor, offset=bias.offset,
                         ap=[[0, 128], *bias.ap])
    nc.sync.dma_start(out=bias_sb, in_=bias_bcast)

    ident = persist.tile([128, 128], BF16, tag="ident", name="ident")
    masks.make_identity(nc, ident[:])

    qTm = persist.tile([128, 8, NL], BF16, tag="qTm", name="qTm")
    kTm = persist.tile([128, 8, N], BF16, tag="kTm", name="kTm")
    wpTm = persist.tile([128, 8, D], BF16, tag="wpTm", name="wpTm")
    attoutT = persist.tile([128, 8, NL], BF16, tag="attoutT", name="attoutT")
    # ones-augmented V: per k-tile [128 ktok, 16 heads, 64 v | 1 ones | pad]
    vv = [persist.tile([128, H, VW], BF16, tag=f"vv{kt}", name=f"vv{kt}")
          for kt in range(16)]
    for kt in range(16):
        nc.gpsimd.memset(vv[kt][:, :, HD:HD + 1], 1.0)

    xt_pool = tc.alloc_tile_pool(name="xt", bufs=1)
    wt_pool = tc.alloc_tile_pool(name="wt", bufs=2)
    xTm = xt_pool.tile([128, 8, NL], BF16, tag="xTm", name="xTm")

    # transient prologue pools on the right stack so they release before the
    # attention pools allocate
    ld_pool = tc.alloc_tile_pool(name="ld", bufs=5, side="right")
    cast_pool = tc.alloc_tile_pool(name="cast", bufs=3, side="right")
    stage_pool = tc.alloc_tile_pool(name="stage", bufs=1, side="right")

    # K^T and V staging share one buffer: cc_k's store drains it before the
    # V projection copies land (WAR tracked by the pool).
    kstage = stage_pool.tile([128, 8, NL], BF16, tag="stage", name="kstage")

    # PSUM: ps (K/V proj) + tr (PE transposes) tags, 2 bufs each = 4 banks
    qkvps = tc.alloc_tile_pool(name="qkv_ps", bufs=2, space="PSUM")

    def load128(src, row0, ldq):
        # 128-row fp32 chunk on queue ldq
        ld = ld_pool.tile([128, D], F32, tag="ld", name="ld")
        ldq.dma_start(out=ld, in_=src[row0:row0 + 128, :])
        return ld

    def cast128(ld, on_dve=False):
        cb = cast_pool.tile([128, D], BF16, tag="cast", name="cb")
        if on_dve:
            nc.vector.tensor_copy(cb, ld)
        else:
            nc.scalar.copy(cb, ld)
        return cb

    def pe_transpose(cb, dstm, r0):
        # transpose a [128, D] bf16 row-tile into dstm[:, :, r0:r0+128] via
        # identity matmuls; one Pool copy moves PSUM -> SBUF
        trp = qkvps.tile([128, 8, 128], BF16, tag="tr", name="trp")
        for c in range(8):
            nc.tensor.matmul(
                out=trp[:, c, :],
                lhsT=cb[:, c * 128:(c + 1) * 128],
                rhs=ident,
                is_transpose=True)
        nc.vector.tensor_copy(dstm[:, :, r0:r0 + 128], trp)

    # x (scalar queue) and w_qkv K rows (sync queue) load concurrently with
    # interleaved buffer rotation; casts follow arrival order on ScalarE.
    # V and Q row loads are emitted behind them on sync; their casts and PE
    # transposes are deferred into the K/V projection phases below.
    x_cb, wk_cb = [], []
    for c in range(8):
        xl = load128(x_l, c * 128, nc.scalar)
        kl = load128(wqkv, D + c * 128, nc.sync)
        x_cb.append(cast128(xl))
        wk_cb.append(cast128(kl))
    wv_ld = [load128(wqkv, 2 * D + c * 128, nc.sync) for c in range(8)]
    wq_ld = [load128(wqkv, c * 128, nc.sync) for c in range(8)]
    # transposes interleave x/wK in the same order as the casts so the cast
    # pool's slot rotation never inverts against the in-order PE queue
    wtK = wt_pool.tile([128, 8, D], BF16, tag="wT", name="wtK")
    for c in range(8):
        pe_transpose(x_cb[c], xTm, c * 128)
        pe_transpose(wk_cb[c], wtK, c * 128)

    # ---- Phase B: K/V projections + collectives -------------------------
    def proj_kv(wt, m, dst, as_lhsT):
        pss = []
        for qc in range(2):
            ps = qkvps.tile([128, 512], F32, tag="ps", name="ps")
            pss.append(ps)
        for k in range(8):
            for qc in range(2):
                if as_lhsT:  # V: out[tok, e]
                    nc.tensor.matmul(
                        out=pss[qc],
                        lhsT=xTm[:, k, m * 128:(m + 1) * 128],
                        rhs=wt[:, k, qc * 512:(qc + 1) * 512],
                        start=(k == 0), stop=(k == 7))
                else:        # K: out[e, tok]
                    nc.tensor.matmul(
                        out=pss[qc],
                        lhsT=wt[:, k, m * 128:(m + 1) * 128],
                        rhs=xTm[:, k, qc * 512:(qc + 1) * 512],
                        start=(k == 0), stop=(k == 7))
        for qc in range(2):
            nc.vector.tensor_copy(dst[:, qc * 512:(qc + 1) * 512], pss[qc])

    # K projection; wV casts+transposes interleave after m=5
    wtV = wt_pool.tile([128, 8, D], BF16, tag="wT", name="wtV")
    for m in range(8):
        proj_kv(wtK, m, kstage[:, m, :], as_lhsT=False)
        if m == 5:
            for c in range(8):
                pe_transpose(cast128(wv_ld[c]), wtV, c * 128)
    # K^T AllGather in two m-halves so attention unit 0 isn't gated on the
    # full 2MB store+gather+load chain
    for kh in range(2):
        cck_dst = bass.AP(tensor=cc_k.tensor,
                          offset=cc_k.offset + kh * 4 * 128 * NL,
                          ap=[[NL, 128], [128 * NL, 4], [1, NL]])
        nc.gpsimd.dma_start(out=cck_dst, in_=kstage[:, 4 * kh:4 * kh + 4, :])
        nc.gpsimd.collective_compute(
            "AllGather", mybir.AluOpType.bypass, replica_groups=RG,
            ins=[cc_k[kh * 512:(kh + 1) * 512, :]], outs=[k_g[kh]])
        for half in range(2):
            src = bass.AP(tensor=k_g.tensor,
                          offset=k_g.offset + (kh * 2 + half) * 512 * NL,
                          ap=[[NL, 128], [128 * NL, 4], [1, NL]])
            nc.sync.dma_start(
                out=kTm[:, 4 * kh:4 * kh + 4, half * NL:(half + 1) * NL],
                in_=src)

    # V projection in four row-quarters; wQ transposes interleave after q=1
    wtQ = wt_pool.tile([128, 8, D], BF16, tag="wT", name="wtQ")
    vstage = stage_pool.tile([128, 8, D], BF16, tag="stage", name="vstage")
    for q in range(4):
        for t in range(2 * q, 2 * q + 2):
            proj_kv(wtV, t, vstage[:, t, :], as_lhsT=True)
        if q == 1:
            for c in range(8):
                pe_transpose(cast128(wq_ld[c]), wtQ, c * 128)
        ccv_dst = bass.AP(tensor=cc_v.tensor, offset=cc_v.offset + q * 256 * D,
                          ap=[[D, 128], [128 * D, 2], [1, D]])
        nc.gpsimd.dma_start(out=ccv_dst, in_=vstage[:, 2 * q:2 * q + 2, :])
        nc.gpsimd.collective_compute(
            "AllGather", mybir.AluOpType.bypass, replica_groups=RG,
            ins=[cc_v[q * 256:(q + 1) * 256, :]],
            outs=[v_g[q]])
        for half in range(2):
            for tl in range(2):
                kt = half * 8 + 2 * q + tl
                nc.sync.dma_start(
                    out=vv[kt][:, :, 0:HD],
                    in_=v_g[q, half, tl * 128:(tl + 1) * 128, :]
                        .rearrange("p (h c) -> p h c", h=H))

    # w_proj round trip (DMA-transpose) runs in the background during
    # attention: loads on sync, casts on DVE, stores gpsimd, transpose sync
    for c in range(8):
        ld = ld_pool.tile([128, D], F32, tag="ld", name="ld")
        nc.sync.dma_start(out=ld, in_=wproj[c * 128:(c + 1) * 128, :])
        cb = cast_pool.tile([128, D], BF16, tag="cast", name="cb")
        nc.vector.tensor_copy(cb, ld)
        nc.gpsimd.dma_start(out=wpbf[c * 128:(c + 1) * 128, :], in_=cb)
    nc.sync.dma_start_transpose(out=wpTm, in_=wpbf)

    qkvps.release()
    stage_pool.release()
    cast_pool.release()
    ld_pool.release()

    # ---- Phase C: Q projection interleaved with attention ---------------
    spool = tc.alloc_tile_pool(name="s_ps", bufs=2, space="PSUM")
    oapool = tc.alloc_tile_pool(name="oa_ps", bufs=1, space="PSUM")
    ptpool = tc.alloc_tile_pool(name="pt", bufs=4)
    stgpool = tc.alloc_tile_pool(name="stg", bufs=2)
    rcpool = tc.alloc_tile_pool(name="rc", bufs=2)
    rbcpool = tc.alloc_tile_pool(name="rbc", bufs=2)
    ytpool = tc.alloc_tile_pool(name="yt", bufs=1)

    def proj_q(m):
        ps = spool.tile([128, 2, 512], F32, tag="s", name="ps_q")
        for k in range(8):
            for qc in range(2):
                nc.tensor.matmul(
                    out=ps[:, qc, :],
                    lhsT=wtQ[:, k, m * 128:(m + 1) * 128],
                    rhs=xTm[:, k, qc * 512:(qc + 1) * 512],
                    start=(k == 0), stop=(k == 7))
        nc.vector.tensor_copy(qTm[:, m, :], ps.rearrange("p a b -> p (a b)"))

    def unit(p, qc):
        # O accumulates in two 8-long even/odd chains per head (shorter PSUM
        # accumulation chains run measurably faster per matmul); the merge
        # happens on DVE right after the stops, freeing the banks early.
        oaE = [oapool.tile([128, 512], F32, tag=f"oaE{h}", name=f"oaE{h}")
               for h in range(2)]
        oaO = [oapool.tile([128, 512], F32, tag=f"oaO{h}", name=f"oaO{h}")
               for h in range(2)]
        for i, kt in enumerate(KT_ORDER):
            s = spool.tile([128, 2, 512], F32, tag="s", name="s")
            for h in range(2):
                nc.tensor.matmul(
                    out=s[:, h, :],
                    lhsT=kTm[h * 64:(h + 1) * 64, p, kt * 128:(kt + 1) * 128],
                    rhs=qTm[h * 64:(h + 1) * 64, p, qc * 512:(qc + 1) * 512],
                    start=True, stop=True,
                    tile_position=(h * 64, 0))
            pt = ptpool.tile([128, 2, 512], BF16, tag="pt", name="pt")
            nc.scalar.activation(pt, s, EXP, scale=SCALE)
            dst = oaE if i % 2 == 0 else oaO
            for h in range(2):
                nc.tensor.matmul(
                    out=dst[h][0:HD + 1, :],
                    lhsT=vv[kt][:, 2 * p + h, 0:HD + 1],
                    rhs=pt[:, h, :],
                    start=(i < 2), stop=(i >= 14))
        # merge even/odd chains: numerator rows -> bf16 staging, denominator
        # row 64 -> f32; then reciprocal, DRAM-bounce broadcast, multiply
        u = qc * 8 + p
        stg = stgpool.tile([128, 2, 512], BF16, tag="stg", name="stg")
        rc = rcpool.tile([1, 2, 512], F32, tag="rc", name="rc")
        dn = rcpool.tile([1, 2, 512], F32, tag="dn", name="dn")
        for h in range(2):
            # DVE reads at most one PSUM operand per instruction
            nc.vector.tensor_copy(stg[0:HD, h, :], oaE[h][0:HD, :])
            nc.vector.tensor_add(stg[0:HD, h, :], stg[0:HD, h, :],
                                 oaO[h][0:HD, :])
            nc.vector.tensor_copy(dn[:, h, :], oaE[h][HD:HD + 1, :])
            nc.vector.tensor_add(dn[:, h, :], dn[:, h, :],
                                 oaO[h][HD:HD + 1, :])
        nc.vector.reciprocal(rc, dn)
        nc.gpsimd.dma_start(out=scratch[u], in_=rc)
        rbc = rbcpool.tile([64, 2, 512], F32, tag="rbc", name="rbc")
        rsrc = bass.AP(tensor=scratch.tensor,
                       offset=scratch.offset + u * 1024,
                       ap=[[0, 64], [512, 2], [1, 512]])
        nc.sync.dma_start(out=rbc, in_=rsrc)
        for h in range(2):
            nc.vector.tensor_mul(
                attoutT[h * 64:(h + 1) * 64, p, qc * 512:(qc + 1) * 512],
                stg[0:HD, h, :], rbc[:, h, :])

    def outproj(tt):
        yt = ytpool.tile([128, D], F32, tag="yt", name="yt")
        ps = spool.tile([128, 2, 512], F32, tag="s", name="ps_o")
        for p in range(8):
            for ec in range(2):
                nc.tensor.matmul(
                    out=ps[:, ec, :],
                    lhsT=attoutT[:, p, tt * 128:(tt + 1) * 128],
                    rhs=wpTm[:, p, ec * 512:(ec + 1) * 512],
                    start=(p == 0), stop=(p == 7))
        for ec in range(2):
            nc.vector.tensor_add(yt[:, ec * 512:(ec + 1) * 512], ps[:, ec, :],
                                 bias_sb[:, ec * 512:(ec + 1) * 512])
        nc.sync.dma_start(out=out[tt * 128:(tt + 1) * 128, :], in_=yt)

    # qc0 pass: Q projection m-tiles lead their consuming unit by two
    proj_q(0)
    proj_q(1)
    for p in range(8):
        if p + 2 < 8:
            proj_q(p + 2)
        unit(p, 0)
    # outproj(qc0 half) emitted two units into the qc1 pass so the PE never
    # waits on the last qc0 unit's normalize round trip
    for p in range(8):
        unit(p, 1)
        if p == 1:
            for tt in range(4):
                outproj(tt)
    for tt in range(4, 8):
        outproj(tt)

    ytpool.release()
    rbcpool.release()
    rcpool.release()
    stgpool.release()
    ptpool.release()
    oapool.release()
    spool.release()
    wt_pool.release()
    xt_pool.release()
    persist.release()


def _build():
    nc = bacc.Bacc("TRN2", target_bir_lowering=False, debug=False,
                   num_devices=NCORES)
    aps = {
        "x_local": nc.dram_tensor("x_local", [NL, D], F32, kind="ExternalInput").ap(),
        "w_qkv": nc.dram_tensor("w_qkv", [3 * D, D], F32, kind="ExternalInput").ap(),
        "w_proj": nc.dram_tensor("w_proj", [D, D], F32, kind="ExternalInput").ap(),
        "b_proj": nc.dram_tensor("b_proj", [D], F32, kind="ExternalInput").ap(),
        "out": nc.dram_tensor("out", [NL, D], F32, kind="ExternalOutput").ap(),
        "wpbf": nc.dram_tensor("wpbf", [D, D], BF16).ap(),
        "cc_k": nc.dram_tensor("cc_k", [D, NL], BF16).ap(),
        "cc_v": nc.dram_tensor("cc_v", [NL, D], BF16).ap(),
        "k_g": nc.dram_tensor("k_g", [2, 2, 512, NL], BF16).ap(),
        "v_g": nc.dram_tensor("v_g", [4, 2, 256, D], BF16).ap(),
        "scratch": nc.dram_tensor("scratch", [16, 2, 512], F32).ap(),
    }
    with tile.TileContext(nc) as tc:
        _emit(tc, aps)
    nc.compile()
    return nc


_NC = None


def _get_nc():
    global _NC
    if _NC is None:
        _NC = _build()
    return _NC


def run(x, w_qkv, w_proj, b_proj, **spmd_kwargs):
    nc = _get_nc()
    x = np.ascontiguousarray(np.asarray(x, dtype=np.float32))
    w_qkv = np.ascontiguousarray(np.asarray(w_qkv, dtype=np.float32))
    w_proj = np.ascontiguousarray(np.asarray(w_proj, dtype=np.float32))
    b_proj = np.ascontiguousarray(np.asarray(b_proj, dtype=np.float32))
    in_maps = []
    for c in range(NCORES):
        b, half = divmod(c, 2)
        in_maps.append({
            "x_local": np.ascontiguousarray(x[b, half * NL:(half + 1) * NL, :]),
            "w_qkv": w_qkv,
            "w_proj": w_proj,
            "b_proj": b_proj,
        })
    res = run_bass_kernel_spmd(nc, in_maps, list(range(NCORES)), **spmd_kwargs)
    y = np.empty((B, N, D), dtype=np.float32)
    for c in range(NCORES):
        b, half = divmod(c, 2)
        y[b, half * NL:(half + 1) * NL, :] = res.results[c]["out"]
    return y, res


def kernel(x, w_qkv, w_proj, b_proj):
    y, _ = run(x, w_qkv, w_proj, b_proj)
    return y
